# revision 10
# baseline (speedup 1.0000x reference)
"""Conv2d (32,128,56,56) x (256,128,3,3) pad=1 -> (32,256,56,56) on 8 trn2 cores.

Strategy: data-parallel over batch (4 images/core). On each core the conv is
9 accumulating matmuls per output tile: contraction over C=128 (partition
dim), stationary operand = per-tap weight slab [C=128, O_half=128], moving
operand = shifted window of the zero-padded input rows [C=128, 8 rows x 56].
PSUM accumulates the 9 taps; DVE adds bias while evacuating to SBUF; DMA out.

Measured facts this build is tuned around (neuron-profile on this pod):
- PE streams 1 col/cycle at ~1.95 GHz under the 8-core power cap (P0
  downclock; dtype-independent), so the matmul stream is ~116 us and the
  only fight is head/tail latency.
- LDWEIGHTS (114 ns bf16) is fully hidden under the 230 ns matmuls.
- exec_time is counted from the first post-preamble instruction to the very
  last drain, so every ns the first matmul starts earlier and the last
  output DMA completes earlier counts.
bf16 operands halve DMA-in bytes; bf16 output halves the DVE evacuation and
DMA-out cost (accumulation is fp32 in PSUM; rel-err ~2.5e-3 << 2e-2 gate).
"""

import os
import sys

for _p in ("/opt/trn_rl_repo", "/root/.axon_site/_ro/trn_rl_repo"):
    if os.path.isdir(_p) and _p not in sys.path:
        sys.path.insert(0, _p)

import numpy as np

N_CORES = 8
B, C, H, W = 32, 128, 56, 56
O, KH, KW = 256, 3, 3
BPC = B // N_CORES          # images per core
HP, WP = H + 2, W + 2       # padded spatial
ROWS = 8                    # output rows per matmul chunk
NCH = H // ROWS             # chunks per image
NF = ROWS * W               # matmul free dim (448 <= 512 fp32 PSUM bank)

N_WARM = 6                  # PE prewarm matmuls (bridge to DMA-ready ~10us)

_cached_nc = None


def _build_program():
    import concourse.tile as tile
    from concourse import bacc, mybir

    nc = bacc.Bacc(
        "TRN2", target_bir_lowering=False, debug=False, num_devices=N_CORES
    )
    f32 = mybir.dt.float32
    fmm = mybir.dt.bfloat16

    xp = nc.dram_tensor("xp", (C, BPC, HP, WP), fmm, kind="ExternalInput").ap()
    wt = nc.dram_tensor("wt", (C, O // C, KH * KW, 128), fmm, kind="ExternalInput").ap()
    bias = nc.dram_tensor("bias", (C, O // C), f32, kind="ExternalInput").ap()
    out = nc.dram_tensor("out", (BPC * O, H * W), fmm, kind="ExternalOutput").ap()

    with tile.TileContext(nc) as tc:
        with (
            tc.tile_pool(name="consts", bufs=1) as consts,
            tc.tile_pool(name="xpool", bufs=1) as xpool,
            tc.tile_pool(name="opool", bufs=16) as opool,
            tc.tile_pool(name="psum", bufs=7, space="PSUM") as pspool,
        ):
            # Small PE prewarm so the PE is busy (and HAM warming) while the
            # critical-path DMAs land; the real matmuls then chain on with no
            # idle gap. Oversizing this delays the real stream 1:1.
            warm_x = consts.tile([C, NF], fmm, tag="warm_x")
            # gpsimd's queue clears the preamble ~1.2us before vector's, so
            # the first warm matmul can issue that much earlier
            nc.gpsimd.memset(warm_x[:], 0.0)
            warm_ps = pspool.tile([128, NF], f32, tag="warm_ps", bufs=1)
            for _ in range(N_WARM):
                nc.tensor.matmul(
                    warm_ps[:], warm_x[:, :128], warm_x[:], start=True, stop=True
                )

            # All loads on the sync HWDGE ring in hand-picked FIFO order: the
            # critical prefix (rows for chunk 0 + the first taps of the oh=0
            # weights) lands first so the matmul stream starts ASAP.
            # Bands are disjoint (no WAR stall against chunk reads); chunk c
            # reads padded rows 8c..8c+9 and may span two bands.
            bands = [(0, 10), (10, 26), (26, 42), (42, HP)]
            w_sb = consts.tile([C, O // C, KH * KW, 128], fmm)
            bias_sb = consts.tile([C, O // C], f32)
            x_sbs = []
            for i in range(BPC):
                x_sb = xpool.tile([C, HP, WP], fmm, tag=f"x{i}")
                x_sbs.append(x_sb)
            # Two parallel HWDGE issue queues (~650ns of descriptor-gen per
            # dma_start each). The critical prefix (chunk-0 rows + oh=0
            # weights) is split across BOTH queues so everything the first
            # matmuls need occupies the first two issue slots of each queue.
            nc.sync.dma_start(x_sbs[0][:, 0:5], xp[:, 0, 0:5])
            nc.scalar.dma_start(x_sbs[0][:, 5:10], xp[:, 0, 5:10])
            nc.sync.dma_start(w_sb[:, 0, 0:5], wt[:, 0, 0:5])
            nc.scalar.dma_start(w_sb[:, 0, 5:9], wt[:, 0, 5:9])
            nc.scalar.dma_start(bias_sb[:], bias[:])
            nc.scalar.dma_start(w_sb[:, 1], wt[:, 1])
            for r0, r1 in bands[1:]:
                nc.sync.dma_start(x_sbs[0][:, r0:r1], xp[:, 0, r0:r1])
            for i in range(1, BPC):
                nc.sync.dma_start(x_sbs[i][:], xp[:, i])

            for i in range(BPC):
                for oh in range(O // C):
                    for ch in range(NCH):
                        y0 = ch * ROWS
                        ps = pspool.tile([128, NF], f32)
                        for t in range(KH * KW):
                            kh, kw = divmod(t, KW)
                            rhs = x_sbs[i][:, y0 + kh : y0 + kh + ROWS, kw : kw + W]
                            lhsT = w_sb[:, oh, t, :]
                            nc.tensor.matmul(
                                ps[:], lhsT, rhs,
                                start=(t == 0), stop=(t == KH * KW - 1),
                            )
                        r0 = i * O + oh * 128
                        last = i == BPC - 1 and oh == O // C - 1 and ch == NCH - 1
                        qs = [nc.sync, nc.scalar]
                        if not last:
                            o_sb = opool.tile([128, NF], fmm)
                            nc.vector.tensor_scalar_add(
                                o_sb[:], ps[:], bias_sb[:, oh : oh + 1]
                            )
                            qs[ch % 2].dma_start(
                                out[r0 : r0 + 128, ch * NF : (ch + 1) * NF], o_sb[:]
                            )
                        else:
                            # split the final evacuation; the two halves issue
                            # on different HWDGE queues so their descriptor
                            # generation overlaps
                            for hx in range(2):
                                o_sb = opool.tile([128, NF // 2], fmm)
                                nc.vector.tensor_scalar_add(
                                    o_sb[:],
                                    ps[:, hx * (NF // 2) : (hx + 1) * (NF // 2)],
                                    bias_sb[:, oh : oh + 1],
                                )
                                c0 = ch * NF + hx * (NF // 2)
                                qs[hx].dma_start(
                                    out[r0 : r0 + 128, c0 : c0 + NF // 2], o_sb[:]
                                )
    nc.compile()
    return nc


def _get_program():
    global _cached_nc
    if _cached_nc is None:
        _cached_nc = _build_program()
    return _cached_nc


def _prep_inputs(x, kernels, biases):
    """Host-side shard + layout prep. Returns list of per-core input maps."""
    import ml_dtypes

    bf16 = ml_dtypes.bfloat16
    x = np.ascontiguousarray(x, dtype=np.float32)
    kernels = np.ascontiguousarray(kernels, dtype=np.float32)
    biases = np.ascontiguousarray(biases, dtype=np.float32)

    xpad = np.zeros((B, C, HP, WP), dtype=bf16)
    xpad[:, :, 1 : H + 1, 1 : W + 1] = x.astype(bf16)

    # wt[c, oh, t, o'] = kernels[oh*128 + o', c, kh, kw]
    wt = np.ascontiguousarray(
        kernels.astype(bf16)
        .transpose(1, 2, 3, 0)
        .reshape(C, KH * KW, O // C, 128)
        .transpose(0, 2, 1, 3)
    )
    # bias_sb[o', h] = biases[h*128 + o']
    bias2 = np.ascontiguousarray(biases.reshape(O // C, C).T)

    in_maps = []
    for core in range(N_CORES):
        xc = np.ascontiguousarray(
            xpad[core * BPC : (core + 1) * BPC].transpose(1, 0, 2, 3)
        )
        in_maps.append({"xp": xc, "wt": wt, "bias": bias2})
    return in_maps


def _run(in_maps, trace=False, **kw):
    from concourse.bass_utils import run_bass_kernel_spmd

    nc = _get_program()
    return run_bass_kernel_spmd(
        nc, in_maps, core_ids=list(range(N_CORES)), trace=trace, **kw
    )


def kernel(x, kernels, biases):
    res = _run(_prep_inputs(x, kernels, biases))
    outs = [
        r["out"].astype(np.float32).reshape(BPC, O, H, W) for r in res.results
    ]
    return np.concatenate(outs, axis=0)
